# revision 2
# baseline (speedup 1.0000x reference)
"""MoE routing kernel for 8 Trainium2 NeuronCores (Bass/Tile, SPMD).

Strategy (expert-parallel, matching the sharding hint):
  - Host computes the gate (softmax + top-2) and dispatches tokens: each of
    the 8 cores owns 2 of the 16 routed experts and receives only the tokens
    routed to its experts (gathered + transposed + zero-padded to a common
    capacity). This is the "all-to-all token dispatch on the topk indices".
  - The output layer (ow) is linear and commutes with the weighted combine,
    so it is folded into each expert's second matmul on the host
    (w2ot = w2[e].T @ ow.T), shrinking stage-2 work by W/OUT = 4x.
  - The shared expert is sharded over its intermediate dim (2048/8=256 rows
    per core); every core computes a partial for all 2048 tokens, also with
    ow folded in.  Bias terms that commute with the output layer
    (b2, sb2, ob) are applied analytically on the host.
  - Device matmuls run in float32r (full-rate fp32 on the PE array).
  - Host combines: scatter-add of combine-weight-scaled routed partials +
    shared partials + analytic bias terms.
"""
import sys

if "/opt/trn_rl_repo" not in sys.path:
    sys.path.insert(0, "/opt/trn_rl_repo")

import numpy as np
import concourse.bass as bass
import concourse.tile as tile
from concourse import mybir
from concourse.bass_utils import run_bass_kernel_spmd

B = 2048
W = 512
E = 16
TOPK = 2
INTER = 1024
SH = 2048
OUT = 128
NCORES = 8
EPC = E // NCORES          # experts per core = 2
SHS = SH // NCORES         # shared-expert inter slice per core = 256
KW = W // 128              # k-tiles over W = 4
MI = INTER // 128          # m-tiles over INTER = 8
MS = SHS // 128            # m-tiles over shared slice = 2
F32 = mybir.dt.float32
F32R = mybir.dt.float32r

# set by test.py to collect a profile; results stashed in LAST_RESULTS
TRACE = False
TRACE_KW = {}
LAST_RESULTS = None


def _legalize_waits(nc):
    """This container's walrus accepts at most 1 sync wait per instruction
    (2 for EventSemaphore).  Hoist excess waits emitted by the Tile
    scheduler into standalone EventSemaphore instructions."""
    for fn in nc.m.functions:
        for blk in fn.blocks:
            out = []
            changed = False
            for inst in blk.instructions:
                si = getattr(inst, "sync_info", None)
                waits = list(si.on_wait) if si is not None and si.on_wait else []
                cap = 2 if isinstance(inst, mybir.InstEventSemaphore) else 1
                if len(waits) > cap:
                    extra, keep = waits[:-cap], waits[-cap:]
                    for i in range(0, len(extra), 2):
                        out.append(mybir.InstEventSemaphore(
                            name=nc.get_next_instruction_name(),
                            engine=inst.engine,
                            ins=[], outs=[],
                            sync_info=mybir.SyncInfo(
                                on_wait=list(extra[i:i + 2]), on_update=[]),
                        ))
                    si.on_wait = keep
                    changed = True
                out.append(inst)
            if changed:
                blk.instructions = out


def _token_chunks(cap):
    """Split [0, cap) into chunks of <=512 (all multiples of 128)."""
    chunks = []
    off = 0
    while off < cap:
        sz = min(512, cap - off)
        chunks.append((off, sz))
        off += sz
    return chunks


def _build_nc(cap):
    """Build the SPMD Bass program for per-expert token capacity `cap`
    (multiple of 128)."""
    nc = bass.Bass("TRN2", target_bir_lowering=False, debug=False)

    def din(name, f, dt=F32R):
        return nc.dram_tensor(name, [128, f], dt, kind="ExternalInput").ap()

    xt = din("xt", KW * B)                   # x.T packed: col block k = x.T[128k:128k+128, :]
    xg = din("xg", EPC * KW * cap)           # gathered tokens per expert, packed like xt
    w1t = din("w1t", EPC * KW * INTER)       # per expert: w1[e].T packed k-blocks
    w3t = din("w3t", EPC * KW * INTER)
    w2ot = din("w2ot", EPC * MI * OUT)       # per expert: (w2[e].T @ ow.T) packed k-blocks
    sw1t = din("sw1t", KW * SHS)             # shared slice: sw1[s].T packed
    sw3t = din("sw3t", KW * SHS)
    sw2ot = din("sw2ot", MS * OUT)           # (sw2[:, s].T @ ow.T) packed
    bias = din("bias", EPC * 2 * MI + 2 * MS, F32)  # b1/b3 per expert (8 cols each), sb1/sb3 (2 cols each)

    yr = nc.dram_tensor("yr", [128, EPC * cap], F32, kind="ExternalOutput").ap()
    zt = nc.dram_tensor("zt", [128, B], F32, kind="ExternalOutput").ap()

    LR = mybir.ActivationFunctionType.Lrelu
    IDT = mybir.ActivationFunctionType.Identity

    with tile.TileContext(nc) as tc:
        with tc.tile_pool(name="wts", bufs=1) as wts, \
             tc.tile_pool(name="work", bufs=2) as work, \
             tc.tile_pool(name="hts", bufs=1) as hts, \
             tc.tile_pool(name="outs", bufs=2) as outs, \
             tc.tile_pool(name="ps", bufs=2, space="PSUM") as ps:

            # ---- resident inputs ----
            bias_t = wts.tile([128, bias.shape[1]], F32, tag="bias")
            nc.sync.dma_start(bias_t[:], bias[:])
            xt_t = wts.tile([128, KW * B], F32R, tag="xt")
            nc.sync.dma_start(xt_t[:], xt[:])
            sw1_t = wts.tile([128, KW * SHS], F32R, tag="sw1")
            nc.sync.dma_start(sw1_t[:], sw1t[:])
            sw3_t = wts.tile([128, KW * SHS], F32R, tag="sw3")
            nc.sync.dma_start(sw3_t[:], sw3t[:])
            sw2_t = wts.tile([128, MS * OUT], F32R, tag="sw2")
            nc.sync.dma_start(sw2_t[:], sw2ot[:])

            def b_ap(col):  # [128,1] per-partition bias column
                return bias_t[:, col:col + 1]

            chunks = _token_chunks(cap)

            # ---------- routed experts ----------
            for e in range(EPC):
                w1_t = work.tile([128, KW * INTER], F32R, tag="w1")
                nc.sync.dma_start(w1_t[:], w1t[:, e * KW * INTER:(e + 1) * KW * INTER])
                w3_t = work.tile([128, KW * INTER], F32R, tag="w3")
                nc.sync.dma_start(w3_t[:], w3t[:, e * KW * INTER:(e + 1) * KW * INTER])
                w2_t = work.tile([128, MI * OUT], F32R, tag="w2")
                nc.sync.dma_start(w2_t[:], w2ot[:, e * MI * OUT:(e + 1) * MI * OUT])
                xg_t = work.tile([128, KW * cap], F32R, tag="xg")
                nc.sync.dma_start(xg_t[:], xg[:, e * KW * cap:(e + 1) * KW * cap])

                for (c0, csz) in chunks:
                    h_tiles = []
                    for m in range(MI):
                        p1 = ps.tile([128, csz], F32, tag="p1")
                        p3 = ps.tile([128, csz], F32, tag="p3")
                        for k in range(KW):
                            lhs1 = w1_t[:, k * INTER + m * 128: k * INTER + (m + 1) * 128]
                            lhs3 = w3_t[:, k * INTER + m * 128: k * INTER + (m + 1) * 128]
                            rhs = xg_t[:, k * cap + c0: k * cap + c0 + csz]
                            nc.tensor.matmul(p1[:], lhs1, rhs, start=(k == 0), stop=(k == KW - 1))
                            nc.tensor.matmul(p3[:], lhs3, rhs, start=(k == 0), stop=(k == KW - 1))
                        a = work.tile([128, csz], F32, tag="act_a")
                        nc.scalar.activation(a[:], p1[:], LR, bias=b_ap(e * 2 * MI + m), alpha=0.01)
                        t3 = work.tile([128, csz], F32, tag="act_b")
                        nc.scalar.activation(t3[:], p3[:], IDT, bias=b_ap(e * 2 * MI + MI + m))
                        ht = hts.tile([128, csz], F32R, tag=f"ht{m}", bufs=3)
                        nc.vector.tensor_mul(ht[:], a[:], t3[:])
                        h_tiles.append(ht)
                    # stage 2: y.T[OUT, tokens] += w2ot_k.T @ h_k
                    py = ps.tile([128, csz], F32, tag="py")
                    for m in range(MI):
                        lhs = w2_t[:, m * OUT:(m + 1) * OUT]
                        nc.tensor.matmul(py[:], lhs, h_tiles[m][:], start=(m == 0), stop=(m == MI - 1))
                    yo = outs.tile([128, csz], F32, tag="yo")
                    nc.vector.tensor_copy(yo[:], py[:])
                    nc.sync.dma_start(yr[:, e * cap + c0: e * cap + c0 + csz], yo[:])

            # ---------- shared expert (inter-sharded slice, all tokens) ----------
            for (c0, csz) in _token_chunks(B):
                hs_tiles = []
                for m in range(MS):
                    p1 = ps.tile([128, csz], F32, tag="p1")
                    p3 = ps.tile([128, csz], F32, tag="p3")
                    for k in range(KW):
                        lhs1 = sw1_t[:, k * SHS + m * 128: k * SHS + (m + 1) * 128]
                        lhs3 = sw3_t[:, k * SHS + m * 128: k * SHS + (m + 1) * 128]
                        rhs = xt_t[:, k * B + c0: k * B + c0 + csz]
                        nc.tensor.matmul(p1[:], lhs1, rhs, start=(k == 0), stop=(k == KW - 1))
                        nc.tensor.matmul(p3[:], lhs3, rhs, start=(k == 0), stop=(k == KW - 1))
                    a = work.tile([128, csz], F32, tag="act_a")
                    nc.scalar.activation(a[:], p1[:], LR, bias=b_ap(EPC * 2 * MI + m), alpha=0.01)
                    t3 = work.tile([128, csz], F32, tag="act_b")
                    nc.scalar.activation(t3[:], p3[:], IDT, bias=b_ap(EPC * 2 * MI + MS + m))
                    hs = hts.tile([128, csz], F32R, tag=f"hs{m}", bufs=2)
                    nc.vector.tensor_mul(hs[:], a[:], t3[:])
                    hs_tiles.append(hs)
                pz = ps.tile([128, csz], F32, tag="py")
                for m in range(MS):
                    lhs = sw2_t[:, m * OUT:(m + 1) * OUT]
                    nc.tensor.matmul(pz[:], lhs, hs_tiles[m][:], start=(m == 0), stop=(m == MS - 1))
                zo = outs.tile([128, csz], F32, tag="zo")
                nc.vector.tensor_copy(zo[:], pz[:])
                nc.sync.dma_start(zt[:, c0:c0 + csz], zo[:])

    _legalize_waits(nc)
    return nc


_NC_CACHE = {}


def _pack_kblocks(mat):
    """[Ktot, F] -> [128, (Ktot/128)*F] with col block k = mat[128k:128(k+1), :]."""
    ktot, f = mat.shape
    assert ktot % 128 == 0
    return np.ascontiguousarray(
        mat.reshape(ktot // 128, 128, f).transpose(1, 0, 2).reshape(128, -1))


def prepare(x, task_id, gate_w, w1, b1, w2, b2, w3, b3,
            sw1, sb1, sw2, sb2, sw3, sb3, ow, ob):
    """Host-side routing + packing.  Returns everything needed to launch the
    device program and combine its partial outputs."""
    x = np.asarray(x, np.float32)
    f32 = lambda a: np.asarray(a, np.float32)
    gate_w, w1, b1, w2, b2, w3, b3 = map(f32, (gate_w, w1, b1, w2, b2, w3, b3))
    sw1, sb1, sw2, sb2, sw3, sb3, ow, ob = map(f32, (sw1, sb1, sw2, sb2, sw3, sb3, ow, ob))

    # ---- host gate: softmax + top-2 (the routing decision) ----
    logits = x @ gate_w.T
    logits -= logits.max(axis=1, keepdims=True)
    ex = np.exp(logits)
    scores = ex / ex.sum(axis=1, keepdims=True)            # [B, E] fp32
    order = np.argsort(-scores, axis=1, kind="stable")[:, :TOPK]   # [B, 2]

    tok_lists = []
    for e in range(E):
        sel = np.nonzero((order == e).any(axis=1))[0]
        tok_lists.append(sel)
    max_cnt = max(len(t) for t in tok_lists)
    cap = max(128, -(-max_cnt // 128) * 128)

    if cap not in _NC_CACHE:
        _NC_CACHE[cap] = _build_nc(cap)
    nc = _NC_CACHE[cap]

    # ---- pack per-core inputs ----
    xt_p = _pack_kblocks(x.T.copy())                       # [128, KW*B]
    in_maps = []
    for c in range(NCORES):
        exps = [c * EPC + j for j in range(EPC)]
        xg_blocks, w1_bl, w3_bl, w2_bl = [], [], [], []
        bias_cols = []
        for e in exps:
            toks = tok_lists[e]
            xge = np.zeros((W, cap), np.float32)
            xge[:, :len(toks)] = x[toks].T
            xg_blocks.append(_pack_kblocks(xge))
            w1_bl.append(_pack_kblocks(w1[e].T.copy()))
            w3_bl.append(_pack_kblocks(w3[e].T.copy()))
            w2_bl.append(_pack_kblocks(w2[e].T @ ow.T))
        for e in exps:
            bias_cols.append(b1[e].reshape(MI, 128).T)     # [128, MI]
            bias_cols.append(b3[e].reshape(MI, 128).T)
        s = slice(c * SHS, (c + 1) * SHS)
        bias_cols.append(sb1[s].reshape(MS, 128).T)
        bias_cols.append(sb3[s].reshape(MS, 128).T)
        in_maps.append({
            "xt": xt_p,
            "xg": np.concatenate(xg_blocks, axis=1),
            "w1t": np.concatenate(w1_bl, axis=1),
            "w3t": np.concatenate(w3_bl, axis=1),
            "w2ot": np.concatenate(w2_bl, axis=1),
            "sw1t": _pack_kblocks(sw1[s].T.copy()),
            "sw3t": _pack_kblocks(sw3[s].T.copy()),
            "sw2ot": _pack_kblocks(sw2[:, s].T @ ow.T),
            "bias": np.ascontiguousarray(np.concatenate(bias_cols, axis=1)),
        })

    # dense combine weights [B, E] (zero except the top-2 experts per token)
    combine_w = np.zeros((B, E), np.float32)
    rows = np.arange(B)
    combine_w[rows[:, None], order] = np.take_along_axis(scores, order, axis=1)
    # analytic bias terms: sum_e combine[:,e] * (b2[e] @ ow.T)  +  sb2 @ ow.T + ob
    base = combine_w @ (b2 @ ow.T) + sb2 @ ow.T + ob

    return dict(nc=nc, cap=cap, in_maps=in_maps, tok_lists=tok_lists,
                combine_w=combine_w, base=base)


def combine(p, results):
    """Combine per-core device partials into the full [B, OUT] output."""
    cap, tok_lists, combine_w = p["cap"], p["tok_lists"], p["combine_w"]
    out = p["base"].astype(np.float32).copy()
    for c in range(NCORES):
        r = results[c]
        out += r["zt"].T
        for j in range(EPC):
            e = c * EPC + j
            toks = tok_lists[e]
            yre = r["yr"][:, j * cap: j * cap + len(toks)]  # [OUT, cnt]
            out[toks] += combine_w[toks, e][:, None] * yre.T
    return out


def kernel(x, task_id, gate_w, w1, b1, w2, b2, w3, b3,
           sw1, sb1, sw2, sb2, sw3, sb3, ow, ob):
    global LAST_RESULTS
    p = prepare(x, task_id, gate_w, w1, b1, w2, b2, w3, b3,
                sw1, sb1, sw2, sb2, sw3, sb3, ow, ob)
    res = run_bass_kernel_spmd(
        p["nc"], p["in_maps"], core_ids=list(range(NCORES)),
        trace=TRACE, **TRACE_KW)
    LAST_RESULTS = res
    return combine(p, res.results)


# revision 3
# speedup vs baseline: 33.2162x; 33.2162x over previous
"""MoE routing kernel for 8 Trainium2 NeuronCores (Bass/Tile, SPMD).

Strategy (expert-parallel, matching the sharding hint):
  - Host computes the gate (softmax + top-2) and dispatches tokens: each of
    the 8 cores owns 2 of the 16 routed experts and receives only the tokens
    routed to its experts (gathered + transposed + zero-padded to a common
    capacity). This is the "all-to-all token dispatch on the topk indices".
  - The output layer (ow) is linear and commutes with the weighted combine,
    so it is folded into each expert's second matmul on the host
    (w2ot = w2[e].T @ ow.T), shrinking stage-2 work by W/OUT = 4x.
  - The shared expert is sharded over its intermediate dim (2048/8=256 rows
    per core); every core computes a partial for all 2048 tokens, also with
    ow folded in.  Bias terms that commute with the output layer
    (b2, sb2, ob) are applied analytically on the host.
  - Device matmuls run in float32r (full-rate fp32 on the PE array).
  - Host combines: scatter-add of combine-weight-scaled routed partials +
    shared partials + analytic bias terms.
"""
import sys

if "/opt/trn_rl_repo" not in sys.path:
    sys.path.insert(0, "/opt/trn_rl_repo")

import numpy as np
import concourse.bass as bass
import concourse.tile as tile
from concourse import mybir
from concourse.bass_utils import run_bass_kernel_spmd

B = 2048
W = 512
E = 16
TOPK = 2
INTER = 1024
SH = 2048
OUT = 128
NCORES = 8
EPC = E // NCORES          # experts per core = 2
SHS = SH // NCORES         # shared-expert inter slice per core = 256
KW = W // 128              # k-tiles over W = 4
MI = INTER // 128          # m-tiles over INTER = 8
MS = SHS // 128            # m-tiles over shared slice = 2
F32 = mybir.dt.float32
F32R = mybir.dt.float32r

# set by test.py to collect a profile; results stashed in LAST_RESULTS
TRACE = False
TRACE_KW = {}
LAST_RESULTS = None


def _legalize_waits(nc):
    """This container's walrus accepts at most 1 sync wait per instruction
    (2 for EventSemaphore).  Hoist excess waits emitted by the Tile
    scheduler into standalone EventSemaphore instructions."""
    for fn in nc.m.functions:
        for blk in fn.blocks:
            out = []
            changed = False
            for inst in blk.instructions:
                si = getattr(inst, "sync_info", None)
                waits = list(si.on_wait) if si is not None and si.on_wait else []
                cap = 2 if isinstance(inst, mybir.InstEventSemaphore) else 1
                if len(waits) > cap:
                    extra, keep = waits[:-cap], waits[-cap:]
                    for i in range(0, len(extra), 2):
                        out.append(mybir.InstEventSemaphore(
                            name=nc.get_next_instruction_name(),
                            engine=inst.engine,
                            ins=[], outs=[],
                            sync_info=mybir.SyncInfo(
                                on_wait=list(extra[i:i + 2]), on_update=[]),
                        ))
                    si.on_wait = keep
                    changed = True
                out.append(inst)
            if changed:
                blk.instructions = out


def _token_chunks(cap):
    """Split [0, cap) into chunks of <=512 (all multiples of 128)."""
    chunks = []
    off = 0
    while off < cap:
        sz = min(512, cap - off)
        chunks.append((off, sz))
        off += sz
    return chunks


def _build_nc(cap, loop_n=None):
    """Build the SPMD Bass program for per-expert token capacity `cap`
    (multiple of 128).  loop_n wraps the body in a hardware For_i loop
    (used only for timing measurements)."""
    nc = bass.Bass("TRN2", target_bir_lowering=False, debug=False)

    def din(name, f, dt=F32R):
        return nc.dram_tensor(name, [128, f], dt, kind="ExternalInput").ap()

    xt = din("xt", KW * B)                   # x.T packed: col block k = x.T[128k:128k+128, :]
    xg = din("xg", EPC * KW * cap)           # gathered tokens per expert, packed like xt
    w1t = din("w1t", EPC * KW * INTER)       # per expert: w1[e].T packed k-blocks
    w3t = din("w3t", EPC * KW * INTER)
    w2ot = din("w2ot", EPC * MI * OUT)       # per expert: (w2[e].T @ ow.T) packed k-blocks
    sw1t = din("sw1t", KW * SHS)             # shared slice: sw1[s].T packed
    sw3t = din("sw3t", KW * SHS)
    sw2ot = din("sw2ot", MS * OUT)           # (sw2[:, s].T @ ow.T) packed
    bias = din("bias", EPC * 2 * MI + 2 * MS, F32)  # b1/b3 per expert (8 cols each), sb1/sb3 (2 cols each)

    yr = nc.dram_tensor("yr", [128, EPC * cap], F32, kind="ExternalOutput").ap()
    zt = nc.dram_tensor("zt", [128, B], F32, kind="ExternalOutput").ap()

    LR = mybir.ActivationFunctionType.Lrelu
    IDT = mybir.ActivationFunctionType.Identity

    with tile.TileContext(nc) as tc:
        import contextlib
        with tc.tile_pool(name="wts", bufs=1) as wts, \
             tc.tile_pool(name="work", bufs=2) as work, \
             tc.tile_pool(name="hts", bufs=1) as hts, \
             tc.tile_pool(name="outs", bufs=2) as outs, \
             tc.tile_pool(name="ps", bufs=2, space="PSUM") as ps, \
             (tc.For_i(0, loop_n, 1) if loop_n is not None else contextlib.nullcontext()):

            # ---- resident inputs ----
            bias_t = wts.tile([128, bias.shape[1]], F32, tag="bias")
            nc.sync.dma_start(bias_t[:], bias[:])
            xt_t = wts.tile([128, KW * B], F32R, tag="xt")
            nc.sync.dma_start(xt_t[:], xt[:])
            sw1_t = wts.tile([128, KW * SHS], F32R, tag="sw1")
            nc.sync.dma_start(sw1_t[:], sw1t[:])
            sw3_t = wts.tile([128, KW * SHS], F32R, tag="sw3")
            nc.sync.dma_start(sw3_t[:], sw3t[:])
            sw2_t = wts.tile([128, MS * OUT], F32R, tag="sw2")
            nc.sync.dma_start(sw2_t[:], sw2ot[:])

            def b_ap(col):  # [128,1] per-partition bias column
                return bias_t[:, col:col + 1]

            chunks = _token_chunks(cap)

            # ---------- routed experts ----------
            for e in range(EPC):
                w1_t = work.tile([128, KW * INTER], F32R, tag="w1")
                nc.sync.dma_start(w1_t[:], w1t[:, e * KW * INTER:(e + 1) * KW * INTER])
                w3_t = work.tile([128, KW * INTER], F32R, tag="w3")
                nc.sync.dma_start(w3_t[:], w3t[:, e * KW * INTER:(e + 1) * KW * INTER])
                w2_t = work.tile([128, MI * OUT], F32R, tag="w2")
                nc.sync.dma_start(w2_t[:], w2ot[:, e * MI * OUT:(e + 1) * MI * OUT])
                xg_t = work.tile([128, KW * cap], F32R, tag="xg")
                nc.sync.dma_start(xg_t[:], xg[:, e * KW * cap:(e + 1) * KW * cap])

                for (c0, csz) in chunks:
                    h_tiles = []
                    for m in range(MI):
                        p1 = ps.tile([128, csz], F32, tag="p1")
                        p3 = ps.tile([128, csz], F32, tag="p3")
                        for k in range(KW):
                            lhs1 = w1_t[:, k * INTER + m * 128: k * INTER + (m + 1) * 128]
                            lhs3 = w3_t[:, k * INTER + m * 128: k * INTER + (m + 1) * 128]
                            rhs = xg_t[:, k * cap + c0: k * cap + c0 + csz]
                            nc.tensor.matmul(p1[:], lhs1, rhs, start=(k == 0), stop=(k == KW - 1))
                            nc.tensor.matmul(p3[:], lhs3, rhs, start=(k == 0), stop=(k == KW - 1))
                        a = work.tile([128, csz], F32, tag="act_a")
                        nc.scalar.activation(a[:], p1[:], LR, bias=b_ap(e * 2 * MI + m), alpha=0.01)
                        t3 = work.tile([128, csz], F32, tag="act_b")
                        nc.scalar.activation(t3[:], p3[:], IDT, bias=b_ap(e * 2 * MI + MI + m))
                        ht = hts.tile([128, csz], F32R, tag=f"ht{m}", bufs=3)
                        nc.vector.tensor_mul(ht[:], a[:], t3[:])
                        h_tiles.append(ht)
                    # stage 2: y.T[OUT, tokens] += w2ot_k.T @ h_k
                    py = ps.tile([128, csz], F32, tag="py")
                    for m in range(MI):
                        lhs = w2_t[:, m * OUT:(m + 1) * OUT]
                        nc.tensor.matmul(py[:], lhs, h_tiles[m][:], start=(m == 0), stop=(m == MI - 1))
                    yo = outs.tile([128, csz], F32, tag="yo")
                    nc.vector.tensor_copy(yo[:], py[:])
                    nc.sync.dma_start(yr[:, e * cap + c0: e * cap + c0 + csz], yo[:])

            # ---------- shared expert (inter-sharded slice, all tokens) ----------
            for (c0, csz) in _token_chunks(B):
                hs_tiles = []
                for m in range(MS):
                    p1 = ps.tile([128, csz], F32, tag="p1")
                    p3 = ps.tile([128, csz], F32, tag="p3")
                    for k in range(KW):
                        lhs1 = sw1_t[:, k * SHS + m * 128: k * SHS + (m + 1) * 128]
                        lhs3 = sw3_t[:, k * SHS + m * 128: k * SHS + (m + 1) * 128]
                        rhs = xt_t[:, k * B + c0: k * B + c0 + csz]
                        nc.tensor.matmul(p1[:], lhs1, rhs, start=(k == 0), stop=(k == KW - 1))
                        nc.tensor.matmul(p3[:], lhs3, rhs, start=(k == 0), stop=(k == KW - 1))
                    a = work.tile([128, csz], F32, tag="act_a")
                    nc.scalar.activation(a[:], p1[:], LR, bias=b_ap(EPC * 2 * MI + m), alpha=0.01)
                    t3 = work.tile([128, csz], F32, tag="act_b")
                    nc.scalar.activation(t3[:], p3[:], IDT, bias=b_ap(EPC * 2 * MI + MS + m))
                    hs = hts.tile([128, csz], F32R, tag=f"hs{m}", bufs=2)
                    nc.vector.tensor_mul(hs[:], a[:], t3[:])
                    hs_tiles.append(hs)
                pz = ps.tile([128, csz], F32, tag="py")
                for m in range(MS):
                    lhs = sw2_t[:, m * OUT:(m + 1) * OUT]
                    nc.tensor.matmul(pz[:], lhs, hs_tiles[m][:], start=(m == 0), stop=(m == MS - 1))
                zo = outs.tile([128, csz], F32, tag="zo")
                nc.vector.tensor_copy(zo[:], pz[:])
                nc.sync.dma_start(zt[:, c0:c0 + csz], zo[:])

    _legalize_waits(nc)
    return nc


_NC_CACHE = {}


def _pack_kblocks(mat):
    """[Ktot, F] -> [128, (Ktot/128)*F] with col block k = mat[128k:128(k+1), :]."""
    ktot, f = mat.shape
    assert ktot % 128 == 0
    return np.ascontiguousarray(
        mat.reshape(ktot // 128, 128, f).transpose(1, 0, 2).reshape(128, -1))


def prepare(x, task_id, gate_w, w1, b1, w2, b2, w3, b3,
            sw1, sb1, sw2, sb2, sw3, sb3, ow, ob):
    """Host-side routing + packing.  Returns everything needed to launch the
    device program and combine its partial outputs."""
    x = np.asarray(x, np.float32)
    f32 = lambda a: np.asarray(a, np.float32)
    gate_w, w1, b1, w2, b2, w3, b3 = map(f32, (gate_w, w1, b1, w2, b2, w3, b3))
    sw1, sb1, sw2, sb2, sw3, sb3, ow, ob = map(f32, (sw1, sb1, sw2, sb2, sw3, sb3, ow, ob))

    # ---- host gate: softmax + top-2 (the routing decision) ----
    logits = x @ gate_w.T
    logits -= logits.max(axis=1, keepdims=True)
    ex = np.exp(logits)
    scores = ex / ex.sum(axis=1, keepdims=True)            # [B, E] fp32
    order = np.argsort(-scores, axis=1, kind="stable")[:, :TOPK]   # [B, 2]

    tok_lists = []
    for e in range(E):
        sel = np.nonzero((order == e).any(axis=1))[0]
        tok_lists.append(sel)
    max_cnt = max(len(t) for t in tok_lists)
    cap = max(128, -(-max_cnt // 128) * 128)

    if cap not in _NC_CACHE:
        _NC_CACHE[cap] = _build_nc(cap)
    nc = _NC_CACHE[cap]

    # ---- pack per-core inputs ----
    xt_p = _pack_kblocks(x.T.copy())                       # [128, KW*B]
    in_maps = []
    for c in range(NCORES):
        exps = [c * EPC + j for j in range(EPC)]
        xg_blocks, w1_bl, w3_bl, w2_bl = [], [], [], []
        bias_cols = []
        for e in exps:
            toks = tok_lists[e]
            xge = np.zeros((W, cap), np.float32)
            xge[:, :len(toks)] = x[toks].T
            xg_blocks.append(_pack_kblocks(xge))
            w1_bl.append(_pack_kblocks(w1[e].T.copy()))
            w3_bl.append(_pack_kblocks(w3[e].T.copy()))
            w2_bl.append(_pack_kblocks(w2[e].T @ ow.T))
        for e in exps:
            bias_cols.append(b1[e].reshape(MI, 128).T)     # [128, MI]
            bias_cols.append(b3[e].reshape(MI, 128).T)
        s = slice(c * SHS, (c + 1) * SHS)
        bias_cols.append(sb1[s].reshape(MS, 128).T)
        bias_cols.append(sb3[s].reshape(MS, 128).T)
        in_maps.append({
            "xt": xt_p,
            "xg": np.concatenate(xg_blocks, axis=1),
            "w1t": np.concatenate(w1_bl, axis=1),
            "w3t": np.concatenate(w3_bl, axis=1),
            "w2ot": np.concatenate(w2_bl, axis=1),
            "sw1t": _pack_kblocks(sw1[s].T.copy()),
            "sw3t": _pack_kblocks(sw3[s].T.copy()),
            "sw2ot": _pack_kblocks(sw2[:, s].T @ ow.T),
            "bias": np.ascontiguousarray(np.concatenate(bias_cols, axis=1)),
        })

    # dense combine weights [B, E] (zero except the top-2 experts per token)
    combine_w = np.zeros((B, E), np.float32)
    rows = np.arange(B)
    combine_w[rows[:, None], order] = np.take_along_axis(scores, order, axis=1)
    # analytic bias terms: sum_e combine[:,e] * (b2[e] @ ow.T)  +  sb2 @ ow.T + ob
    base = combine_w @ (b2 @ ow.T) + sb2 @ ow.T + ob

    return dict(nc=nc, cap=cap, in_maps=in_maps, tok_lists=tok_lists,
                combine_w=combine_w, base=base)


def combine(p, results):
    """Combine per-core device partials into the full [B, OUT] output."""
    cap, tok_lists, combine_w = p["cap"], p["tok_lists"], p["combine_w"]
    out = p["base"].astype(np.float32).copy()
    for c in range(NCORES):
        r = results[c]
        out += r["zt"].T
        for j in range(EPC):
            e = c * EPC + j
            toks = tok_lists[e]
            yre = r["yr"][:, j * cap: j * cap + len(toks)]  # [OUT, cnt]
            out[toks] += combine_w[toks, e][:, None] * yre.T
    return out


def kernel(x, task_id, gate_w, w1, b1, w2, b2, w3, b3,
           sw1, sb1, sw2, sb2, sw3, sb3, ow, ob):
    global LAST_RESULTS
    p = prepare(x, task_id, gate_w, w1, b1, w2, b2, w3, b3,
                sw1, sb1, sw2, sb2, sw3, sb3, ow, ob)
    res = run_bass_kernel_spmd(
        p["nc"], p["in_maps"], core_ids=list(range(NCORES)),
        trace=TRACE, **TRACE_KW)
    LAST_RESULTS = res
    return combine(p, res.results)


# revision 10
# speedup vs baseline: 36.3602x; 1.0947x over previous
"""MoE routing kernel for 8 Trainium2 NeuronCores (Bass/Tile, SPMD).

Strategy (expert-parallel, matching the sharding hint):
  - Host computes the gate (softmax + top-2) and dispatches tokens: each of
    the 8 cores owns 2 of the 16 routed experts and receives only the tokens
    routed to its experts (gathered + transposed + zero-padded to a common
    capacity). This is the "all-to-all token dispatch on the topk indices".
  - The output layer (ow) is linear and commutes with the weighted combine,
    so it is folded into each expert's second matmul on the host
    (w2ot = w2[e].T @ ow.T), shrinking stage-2 work by W/OUT = 4x.
  - The shared expert is sharded over its intermediate dim (2048/8=256 rows
    per core); every core computes a partial for all 2048 tokens, also with
    ow folded in.  Bias terms that commute with the output layer
    (b2, sb2, ob) are applied analytically on the host.
  - Device matmuls run in float32r (full-rate fp32 on the PE array).
  - Host combines: scatter-add of combine-weight-scaled routed partials +
    shared partials + analytic bias terms.
"""
import sys

if "/opt/trn_rl_repo" not in sys.path:
    sys.path.insert(0, "/opt/trn_rl_repo")

import numpy as np
import concourse.bass as bass
import concourse.tile as tile
from concourse import mybir
from concourse.bass_utils import run_bass_kernel_spmd

B = 2048
W = 512
E = 16
TOPK = 2
INTER = 1024
SH = 2048
OUT = 128
NCORES = 8
EPC = E // NCORES          # experts per core = 2
SHS = SH // NCORES         # shared-expert inter slice per core = 256
KW = W // 128              # k-tiles over W = 4
MI = INTER // 128          # m-tiles over INTER = 8
MS = SHS // 128            # m-tiles over shared slice = 2
F32 = mybir.dt.float32
F32R = mybir.dt.float32r

# set by test.py to collect a profile; results stashed in LAST_RESULTS
TRACE = False
TRACE_KW = {}
LAST_RESULTS = None


def _legalize_waits(nc):
    """This container's walrus accepts at most 1 sync wait per instruction
    (2 for EventSemaphore).  Hoist excess waits emitted by the Tile
    scheduler into standalone EventSemaphore instructions."""
    for fn in nc.m.functions:
        for blk in fn.blocks:
            out = []
            changed = False
            for inst in blk.instructions:
                si = getattr(inst, "sync_info", None)
                waits = list(si.on_wait) if si is not None and si.on_wait else []
                cap = 2 if isinstance(inst, mybir.InstEventSemaphore) else 1
                if len(waits) > cap:
                    extra, keep = waits[:-cap], waits[-cap:]
                    for i in range(0, len(extra), 2):
                        out.append(mybir.InstEventSemaphore(
                            name=nc.get_next_instruction_name(),
                            engine=inst.engine,
                            ins=[], outs=[],
                            sync_info=mybir.SyncInfo(
                                on_wait=list(extra[i:i + 2]), on_update=[]),
                        ))
                    si.on_wait = keep
                    changed = True
                out.append(inst)
            if changed:
                blk.instructions = out


def _token_chunks(cap):
    """Split [0, cap) into chunks of <=512 (all multiples of 128)."""
    chunks = []
    off = 0
    while off < cap:
        sz = min(512, cap - off)
        chunks.append((off, sz))
        off += sz
    return chunks


def _build_nc(cap, loop_n=None, legalize=True):
    """Build the SPMD Bass program for per-expert token capacity `cap`
    (multiple of 128).  loop_n wraps the body in a hardware For_i loop
    (used only for timing measurements)."""
    nc = bass.Bass("TRN2", target_bir_lowering=False, debug=False)

    def din(name, f, dt=F32R):
        return nc.dram_tensor(name, [128, f], dt, kind="ExternalInput").ap()

    xt = din("xt", KW * B)                   # x.T packed: col block k = x.T[128k:128k+128, :]
    xg = din("xg", EPC * KW * cap)           # gathered tokens per expert, packed like xt
    w1t = din("w1t", EPC * KW * INTER)       # per expert: w1[e].T packed k-blocks
    w3t = din("w3t", EPC * KW * INTER)
    w2ot = din("w2ot", EPC * MI * OUT)       # per expert: (w2[e].T @ ow.T) packed k-blocks
    sw1t = din("sw1t", KW * SHS)             # shared slice: sw1[s].T packed
    sw3t = din("sw3t", KW * SHS)
    sw2ot = din("sw2ot", MS * OUT)           # (sw2[:, s].T @ ow.T) packed
    bias = din("bias", EPC * 2 * MI + 2 * MS, F32)  # b1/b3 per expert (8 cols each), sb1/sb3 (2 cols each)

    yr = nc.dram_tensor("yr", [128, EPC * cap], F32, kind="ExternalOutput").ap()
    zt = nc.dram_tensor("zt", [128, B], F32, kind="ExternalOutput").ap()

    LR = mybir.ActivationFunctionType.Lrelu
    IDT = mybir.ActivationFunctionType.Identity

    with tile.TileContext(nc) as tc:
        import contextlib
        with tc.tile_pool(name="wts", bufs=1) as wts, \
             tc.tile_pool(name="work", bufs=2) as work, \
             tc.tile_pool(name="hts", bufs=1) as hts, \
             tc.tile_pool(name="outs", bufs=2) as outs, \
             tc.tile_pool(name="ps", bufs=2, space="PSUM") as ps, \
             (tc.For_i(0, loop_n, 1) if loop_n is not None else contextlib.nullcontext()):

            # ---- tiny bias + shared-expert inputs go on otherwise-idle DMA
            # queues (Pool/ACT) so the SP queue is dedicated to expert weights.
            bias_t = wts.tile([128, bias.shape[1]], F32, tag="bias")
            nc.scalar.dma_start(bias_t[:], bias[:])
            xt_ts = []
            for k in range(KW):
                t = wts.tile([128, B], F32R, tag=f"xtk{k}")
                nc.scalar.dma_start(t[:], xt[:, k * B:(k + 1) * B])
                xt_ts.append(t)
            sw1_t = wts.tile([128, KW * SHS], F32R, tag="sw1")
            nc.scalar.dma_start(sw1_t[:], sw1t[:])
            sw3_t = wts.tile([128, KW * SHS], F32R, tag="sw3")
            nc.scalar.dma_start(sw3_t[:], sw3t[:])
            sw2_t = wts.tile([128, MS * OUT], F32R, tag="sw2")
            nc.scalar.dma_start(sw2_t[:], sw2ot[:])

            def b_ap(col):  # [128,1] per-partition bias column
                return bias_t[:, col:col + 1]

            chunks = _token_chunks(cap)

            def expert_block(e):
                # per-k-tile weight/act loads so the first matmul only waits
                # on the first 0.5 MB, not the whole 4 MB
                w1_ts, w3_ts, xg_ts = [], [], []
                for k in range(KW):
                    t = work.tile([128, INTER], F32R, tag=f"w1k{k}")
                    nc.sync.dma_start(t[:], w1t[:, (e * KW + k) * INTER:(e * KW + k + 1) * INTER])
                    w1_ts.append(t)
                    t = work.tile([128, INTER], F32R, tag=f"w3k{k}")
                    nc.sync.dma_start(t[:], w3t[:, (e * KW + k) * INTER:(e * KW + k + 1) * INTER])
                    w3_ts.append(t)
                    t = work.tile([128, cap], F32R, tag=f"xgk{k}")
                    nc.sync.dma_start(t[:], xg[:, (e * KW + k) * cap:(e * KW + k + 1) * cap])
                    xg_ts.append(t)
                w2_t = work.tile([128, MI * OUT], F32R, tag="w2")
                nc.sync.dma_start(w2_t[:], w2ot[:, e * MI * OUT:(e + 1) * MI * OUT])

                for (c0, csz) in chunks:
                    h_tiles = []
                    for m in range(MI):
                        p1 = ps.tile([128, csz], F32, tag="p1", bufs=3)
                        p3 = ps.tile([128, csz], F32, tag="p3", bufs=3)
                        for k in range(KW):
                            lhs1 = w1_ts[k][:, m * 128:(m + 1) * 128]
                            lhs3 = w3_ts[k][:, m * 128:(m + 1) * 128]
                            rhs = xg_ts[k][:, c0:c0 + csz]
                            nc.tensor.matmul(p1[:], lhs1, rhs, start=(k == 0), stop=(k == KW - 1))
                            nc.tensor.matmul(p3[:], lhs3, rhs, start=(k == 0), stop=(k == KW - 1))
                        a = work.tile([128, csz], F32, tag="act_a")
                        nc.scalar.activation(a[:], p1[:], LR, bias=b_ap(e * 2 * MI + m), alpha=0.01)
                        t3 = work.tile([128, csz], F32, tag="act_b")
                        nc.vector.tensor_scalar_add(t3[:], p3[:], b_ap(e * 2 * MI + MI + m))
                        ht = hts.tile([128, csz], F32R, tag=f"ht{m}", bufs=2)
                        nc.vector.tensor_mul(ht[:], a[:], t3[:])
                        h_tiles.append(ht)
                    # stage 2: y.T[OUT, tokens] += w2ot_k.T @ h_k
                    py = ps.tile([128, csz], F32, tag="py")
                    for m in range(MI):
                        lhs = w2_t[:, m * OUT:(m + 1) * OUT]
                        nc.tensor.matmul(py[:], lhs, h_tiles[m][:], start=(m == 0), stop=(m == MI - 1))
                    yo = outs.tile([128, csz], F32, tag="yo")
                    nc.scalar.copy(yo[:], py[:])
                    nc.sync.dma_start(yr[:, e * cap + c0: e * cap + c0 + csz], yo[:])

            def shared_group(c0, csz):
                hs_tiles = []
                for m in range(MS):
                    p1 = ps.tile([128, csz], F32, tag="p1", bufs=3)
                    p3 = ps.tile([128, csz], F32, tag="p3", bufs=3)
                    for k in range(KW):
                        lhs1 = sw1_t[:, k * SHS + m * 128: k * SHS + (m + 1) * 128]
                        lhs3 = sw3_t[:, k * SHS + m * 128: k * SHS + (m + 1) * 128]
                        rhs = xt_ts[k][:, c0:c0 + csz]
                        nc.tensor.matmul(p1[:], lhs1, rhs, start=(k == 0), stop=(k == KW - 1))
                        nc.tensor.matmul(p3[:], lhs3, rhs, start=(k == 0), stop=(k == KW - 1))
                    a = work.tile([128, csz], F32, tag="act_a")
                    nc.scalar.activation(a[:], p1[:], LR, bias=b_ap(EPC * 2 * MI + m), alpha=0.01)
                    t3 = work.tile([128, csz], F32, tag="act_b")
                    nc.vector.tensor_scalar_add(t3[:], p3[:], b_ap(EPC * 2 * MI + MS + m))
                    hs = hts.tile([128, csz], F32R, tag=f"hs{m}", bufs=2)
                    nc.vector.tensor_mul(hs[:], a[:], t3[:])
                    hs_tiles.append(hs)
                pz = ps.tile([128, csz], F32, tag="py")
                for m in range(MS):
                    lhs = sw2_t[:, m * OUT:(m + 1) * OUT]
                    nc.tensor.matmul(pz[:], lhs, hs_tiles[m][:], start=(m == 0), stop=(m == MS - 1))
                zo = outs.tile([128, csz], F32, tag="zo")
                nc.scalar.copy(zo[:], pz[:])
                nc.sync.dma_start(zt[:, c0:c0 + csz], zo[:])

            # interleave: expert 0, shared half, expert 1, shared half —
            # shared-expert matmuls fill the PE while expert-1 weights stream.
            shared_chunks = _token_chunks(B)
            nsh = len(shared_chunks)
            expert_block(0)
            for (c0, csz) in shared_chunks[:nsh // 2]:
                shared_group(c0, csz)
            for e in range(1, EPC):
                expert_block(e)
            for (c0, csz) in shared_chunks[nsh // 2:]:
                shared_group(c0, csz)

    if legalize:
        _legalize_waits(nc)
    return nc


_NC_CACHE = {}


def _pack_kblocks(mat):
    """[Ktot, F] -> [128, (Ktot/128)*F] with col block k = mat[128k:128(k+1), :]."""
    ktot, f = mat.shape
    assert ktot % 128 == 0
    return np.ascontiguousarray(
        mat.reshape(ktot // 128, 128, f).transpose(1, 0, 2).reshape(128, -1))


def prepare(x, task_id, gate_w, w1, b1, w2, b2, w3, b3,
            sw1, sb1, sw2, sb2, sw3, sb3, ow, ob):
    """Host-side routing + packing.  Returns everything needed to launch the
    device program and combine its partial outputs."""
    x = np.asarray(x, np.float32)
    f32 = lambda a: np.asarray(a, np.float32)
    gate_w, w1, b1, w2, b2, w3, b3 = map(f32, (gate_w, w1, b1, w2, b2, w3, b3))
    sw1, sb1, sw2, sb2, sw3, sb3, ow, ob = map(f32, (sw1, sb1, sw2, sb2, sw3, sb3, ow, ob))

    # ---- host gate: softmax + top-2 (the routing decision) ----
    logits = x @ gate_w.T
    logits -= logits.max(axis=1, keepdims=True)
    ex = np.exp(logits)
    scores = ex / ex.sum(axis=1, keepdims=True)            # [B, E] fp32
    order = np.argsort(-scores, axis=1, kind="stable")[:, :TOPK]   # [B, 2]

    tok_lists = []
    for e in range(E):
        sel = np.nonzero((order == e).any(axis=1))[0]
        tok_lists.append(sel)
    max_cnt = max(len(t) for t in tok_lists)
    cap = max(128, -(-max_cnt // 16) * 16)

    if cap not in _NC_CACHE:
        _NC_CACHE[cap] = _build_nc(cap)
    nc = _NC_CACHE[cap]

    # ---- pack per-core inputs ----
    xt_p = _pack_kblocks(x.T.copy())                       # [128, KW*B]
    in_maps = []
    for c in range(NCORES):
        exps = [c * EPC + j for j in range(EPC)]
        xg_blocks, w1_bl, w3_bl, w2_bl = [], [], [], []
        bias_cols = []
        for e in exps:
            toks = tok_lists[e]
            xge = np.zeros((W, cap), np.float32)
            xge[:, :len(toks)] = x[toks].T
            xg_blocks.append(_pack_kblocks(xge))
            w1_bl.append(_pack_kblocks(w1[e].T.copy()))
            w3_bl.append(_pack_kblocks(w3[e].T.copy()))
            w2_bl.append(_pack_kblocks(w2[e].T @ ow.T))
        for e in exps:
            bias_cols.append(b1[e].reshape(MI, 128).T)     # [128, MI]
            bias_cols.append(b3[e].reshape(MI, 128).T)
        s = slice(c * SHS, (c + 1) * SHS)
        bias_cols.append(sb1[s].reshape(MS, 128).T)
        bias_cols.append(sb3[s].reshape(MS, 128).T)
        in_maps.append({
            "xt": xt_p,
            "xg": np.concatenate(xg_blocks, axis=1),
            "w1t": np.concatenate(w1_bl, axis=1),
            "w3t": np.concatenate(w3_bl, axis=1),
            "w2ot": np.concatenate(w2_bl, axis=1),
            "sw1t": _pack_kblocks(sw1[s].T.copy()),
            "sw3t": _pack_kblocks(sw3[s].T.copy()),
            "sw2ot": _pack_kblocks(sw2[:, s].T @ ow.T),
            "bias": np.ascontiguousarray(np.concatenate(bias_cols, axis=1)),
        })

    # dense combine weights [B, E] (zero except the top-2 experts per token)
    combine_w = np.zeros((B, E), np.float32)
    rows = np.arange(B)
    combine_w[rows[:, None], order] = np.take_along_axis(scores, order, axis=1)
    # analytic bias terms: sum_e combine[:,e] * (b2[e] @ ow.T)  +  sb2 @ ow.T + ob
    base = combine_w @ (b2 @ ow.T) + sb2 @ ow.T + ob

    return dict(nc=nc, cap=cap, in_maps=in_maps, tok_lists=tok_lists,
                combine_w=combine_w, base=base)


def combine(p, results):
    """Combine per-core device partials into the full [B, OUT] output."""
    cap, tok_lists, combine_w = p["cap"], p["tok_lists"], p["combine_w"]
    out = p["base"].astype(np.float32).copy()
    for c in range(NCORES):
        r = results[c]
        out += r["zt"].T
        for j in range(EPC):
            e = c * EPC + j
            toks = tok_lists[e]
            yre = r["yr"][:, j * cap: j * cap + len(toks)]  # [OUT, cnt]
            out[toks] += combine_w[toks, e][:, None] * yre.T
    return out


def kernel(x, task_id, gate_w, w1, b1, w2, b2, w3, b3,
           sw1, sb1, sw2, sb2, sw3, sb3, ow, ob):
    global LAST_RESULTS
    p = prepare(x, task_id, gate_w, w1, b1, w2, b2, w3, b3,
                sw1, sb1, sw2, sb2, sw3, sb3, ow, ob)
    res = run_bass_kernel_spmd(
        p["nc"], p["in_maps"], core_ids=list(range(NCORES)),
        trace=TRACE, **TRACE_KW)
    LAST_RESULTS = res
    return combine(p, res.results)


# revision 11
# speedup vs baseline: 53.8566x; 1.4812x over previous
"""MoE routing kernel for 8 Trainium2 NeuronCores (Bass/Tile, SPMD).

Strategy (expert-parallel, matching the sharding hint):
  - Host computes the gate (softmax + top-2) and dispatches tokens: each of
    the 8 cores owns 2 of the 16 routed experts and receives only the tokens
    routed to its experts (gathered + transposed + zero-padded to a common
    capacity). This is the "all-to-all token dispatch on the topk indices".
  - The output layer (ow) is linear and commutes with the weighted combine,
    so it is folded into each expert's second matmul on the host
    (w2ot = w2[e].T @ ow.T), shrinking stage-2 work by W/OUT = 4x.
  - The shared expert is sharded over its intermediate dim (2048/8=256 rows
    per core); every core computes a partial for all 2048 tokens, also with
    ow folded in.  Bias terms that commute with the output layer
    (b2, sb2, ob) are applied analytically on the host.
  - Device matmuls run in float32r (full-rate fp32 on the PE array).
  - Host combines: scatter-add of combine-weight-scaled routed partials +
    shared partials + analytic bias terms.
"""
import sys

if "/opt/trn_rl_repo" not in sys.path:
    sys.path.insert(0, "/opt/trn_rl_repo")

import numpy as np
import concourse.bass as bass
import concourse.tile as tile
from concourse import mybir
from concourse.bass_utils import run_bass_kernel_spmd

B = 2048
W = 512
E = 16
TOPK = 2
INTER = 1024
SH = 2048
OUT = 128
NCORES = 8
EPC = E // NCORES          # experts per core = 2
SHS = SH // NCORES         # shared-expert inter slice per core = 256
KW = W // 128              # k-tiles over W = 4
MI = INTER // 128          # m-tiles over INTER = 8
MS = SHS // 128            # m-tiles over shared slice = 2
F32 = mybir.dt.float32
F32R = mybir.dt.float32r
F16 = mybir.dt.float16
DT = F16                   # device datapath dtype for matmul operands
NPDT = np.float16

# set by test.py to collect a profile; results stashed in LAST_RESULTS
TRACE = False
TRACE_KW = {}
LAST_RESULTS = None


def _legalize_waits(nc):
    """This container's walrus accepts at most 1 sync wait per instruction
    (2 for EventSemaphore).  Hoist excess waits emitted by the Tile
    scheduler into standalone EventSemaphore instructions."""
    for fn in nc.m.functions:
        for blk in fn.blocks:
            out = []
            changed = False
            for inst in blk.instructions:
                si = getattr(inst, "sync_info", None)
                waits = list(si.on_wait) if si is not None and si.on_wait else []
                cap = 2 if isinstance(inst, mybir.InstEventSemaphore) else 1
                if len(waits) > cap:
                    extra, keep = waits[:-cap], waits[-cap:]
                    for i in range(0, len(extra), 2):
                        out.append(mybir.InstEventSemaphore(
                            name=nc.get_next_instruction_name(),
                            engine=inst.engine,
                            ins=[], outs=[],
                            sync_info=mybir.SyncInfo(
                                on_wait=list(extra[i:i + 2]), on_update=[]),
                        ))
                    si.on_wait = keep
                    changed = True
                out.append(inst)
            if changed:
                blk.instructions = out


def _token_chunks(cap):
    """Split [0, cap) into chunks of <=512 (all multiples of 128)."""
    chunks = []
    off = 0
    while off < cap:
        sz = min(512, cap - off)
        chunks.append((off, sz))
        off += sz
    return chunks


def _build_nc(cap, loop_n=None, legalize=True):
    """Build the SPMD Bass program for per-expert token capacity `cap`
    (multiple of 128).  loop_n wraps the body in a hardware For_i loop
    (used only for timing measurements)."""
    nc = bass.Bass("TRN2", target_bir_lowering=False, debug=False)

    def din(name, f, dt=DT):
        return nc.dram_tensor(name, [128, f], dt, kind="ExternalInput").ap()

    xt = din("xt", KW * B)                   # x.T packed: col block k = x.T[128k:128k+128, :]
    xg = din("xg", EPC * KW * cap)           # gathered tokens per expert, packed like xt
    w1t = din("w1t", EPC * KW * INTER)       # per expert: w1[e].T packed k-blocks
    w3t = din("w3t", EPC * KW * INTER)
    w2ot = din("w2ot", EPC * MI * OUT)       # per expert: (w2[e].T @ ow.T) packed k-blocks
    sw1t = din("sw1t", KW * SHS)             # shared slice: sw1[s].T packed
    sw3t = din("sw3t", KW * SHS)
    sw2ot = din("sw2ot", MS * OUT)           # (sw2[:, s].T @ ow.T) packed
    bias = din("bias", EPC * 2 * MI + 2 * MS, F32)  # b1/b3 per expert (8 cols each), sb1/sb3 (2 cols each)

    yr = nc.dram_tensor("yr", [128, EPC * cap], F32, kind="ExternalOutput").ap()
    zt = nc.dram_tensor("zt", [128, B], F32, kind="ExternalOutput").ap()

    LR = mybir.ActivationFunctionType.Lrelu
    IDT = mybir.ActivationFunctionType.Identity

    with tile.TileContext(nc) as tc:
        import contextlib
        with tc.tile_pool(name="wts", bufs=1) as wts, \
             tc.tile_pool(name="work", bufs=2) as work, \
             tc.tile_pool(name="hts", bufs=1) as hts, \
             tc.tile_pool(name="outs", bufs=2) as outs, \
             tc.tile_pool(name="ps", bufs=2, space="PSUM") as ps, \
             (tc.For_i(0, loop_n, 1) if loop_n is not None else contextlib.nullcontext()):

            # ---- tiny bias + shared-expert inputs go on otherwise-idle DMA
            # queues (Pool/ACT) so the SP queue is dedicated to expert weights.
            bias_t = wts.tile([128, bias.shape[1]], F32, tag="bias")
            nc.scalar.dma_start(bias_t[:], bias[:])
            xt_ts = []
            for k in range(KW):
                t = wts.tile([128, B], DT, tag=f"xtk{k}")
                nc.scalar.dma_start(t[:], xt[:, k * B:(k + 1) * B])
                xt_ts.append(t)
            sw1_t = wts.tile([128, KW * SHS], DT, tag="sw1")
            nc.scalar.dma_start(sw1_t[:], sw1t[:])
            sw3_t = wts.tile([128, KW * SHS], DT, tag="sw3")
            nc.scalar.dma_start(sw3_t[:], sw3t[:])
            sw2_t = wts.tile([128, MS * OUT], DT, tag="sw2")
            nc.scalar.dma_start(sw2_t[:], sw2ot[:])

            def b_ap(col):  # [128,1] per-partition bias column
                return bias_t[:, col:col + 1]

            chunks = _token_chunks(cap)

            def expert_block(e):
                # per-k-tile weight/act loads so the first matmul only waits
                # on the first 0.5 MB, not the whole 4 MB
                w1_ts, w3_ts, xg_ts = [], [], []
                for k in range(KW):
                    t = work.tile([128, INTER], DT, tag=f"w1k{k}")
                    nc.sync.dma_start(t[:], w1t[:, (e * KW + k) * INTER:(e * KW + k + 1) * INTER])
                    w1_ts.append(t)
                    t = work.tile([128, INTER], DT, tag=f"w3k{k}")
                    nc.sync.dma_start(t[:], w3t[:, (e * KW + k) * INTER:(e * KW + k + 1) * INTER])
                    w3_ts.append(t)
                    t = work.tile([128, cap], DT, tag=f"xgk{k}")
                    nc.sync.dma_start(t[:], xg[:, (e * KW + k) * cap:(e * KW + k + 1) * cap])
                    xg_ts.append(t)
                w2_t = work.tile([128, MI * OUT], DT, tag="w2")
                nc.sync.dma_start(w2_t[:], w2ot[:, e * MI * OUT:(e + 1) * MI * OUT])

                for (c0, csz) in chunks:
                    h_tiles = []
                    for m in range(MI):
                        p1 = ps.tile([128, csz], F32, tag="p1", bufs=3)
                        p3 = ps.tile([128, csz], F32, tag="p3", bufs=3)
                        for k in range(KW):
                            lhs1 = w1_ts[k][:, m * 128:(m + 1) * 128]
                            lhs3 = w3_ts[k][:, m * 128:(m + 1) * 128]
                            rhs = xg_ts[k][:, c0:c0 + csz]
                            nc.tensor.matmul(p1[:], lhs1, rhs, start=(k == 0), stop=(k == KW - 1))
                            nc.tensor.matmul(p3[:], lhs3, rhs, start=(k == 0), stop=(k == KW - 1))
                        a = work.tile([128, csz], F32, tag="act_a")
                        nc.scalar.activation(a[:], p1[:], LR, bias=b_ap(e * 2 * MI + m), alpha=0.01)
                        t3 = work.tile([128, csz], F32, tag="act_b")
                        nc.vector.tensor_scalar_add(t3[:], p3[:], b_ap(e * 2 * MI + MI + m))
                        ht = hts.tile([128, csz], DT, tag=f"ht{m}", bufs=2)
                        nc.vector.tensor_mul(ht[:], a[:], t3[:])
                        h_tiles.append(ht)
                    # stage 2: y.T[OUT, tokens] += w2ot_k.T @ h_k
                    py = ps.tile([128, csz], F32, tag="py")
                    for m in range(MI):
                        lhs = w2_t[:, m * OUT:(m + 1) * OUT]
                        nc.tensor.matmul(py[:], lhs, h_tiles[m][:], start=(m == 0), stop=(m == MI - 1))
                    yo = outs.tile([128, csz], F32, tag="yo")
                    nc.scalar.copy(yo[:], py[:])
                    nc.sync.dma_start(yr[:, e * cap + c0: e * cap + c0 + csz], yo[:])

            def shared_group(c0, csz):
                hs_tiles = []
                for m in range(MS):
                    p1 = ps.tile([128, csz], F32, tag="p1", bufs=3)
                    p3 = ps.tile([128, csz], F32, tag="p3", bufs=3)
                    for k in range(KW):
                        lhs1 = sw1_t[:, k * SHS + m * 128: k * SHS + (m + 1) * 128]
                        lhs3 = sw3_t[:, k * SHS + m * 128: k * SHS + (m + 1) * 128]
                        rhs = xt_ts[k][:, c0:c0 + csz]
                        nc.tensor.matmul(p1[:], lhs1, rhs, start=(k == 0), stop=(k == KW - 1))
                        nc.tensor.matmul(p3[:], lhs3, rhs, start=(k == 0), stop=(k == KW - 1))
                    a = work.tile([128, csz], F32, tag="act_a")
                    nc.scalar.activation(a[:], p1[:], LR, bias=b_ap(EPC * 2 * MI + m), alpha=0.01)
                    t3 = work.tile([128, csz], F32, tag="act_b")
                    nc.vector.tensor_scalar_add(t3[:], p3[:], b_ap(EPC * 2 * MI + MS + m))
                    hs = hts.tile([128, csz], DT, tag=f"hs{m}", bufs=2)
                    nc.vector.tensor_mul(hs[:], a[:], t3[:])
                    hs_tiles.append(hs)
                pz = ps.tile([128, csz], F32, tag="py")
                for m in range(MS):
                    lhs = sw2_t[:, m * OUT:(m + 1) * OUT]
                    nc.tensor.matmul(pz[:], lhs, hs_tiles[m][:], start=(m == 0), stop=(m == MS - 1))
                zo = outs.tile([128, csz], F32, tag="zo")
                nc.scalar.copy(zo[:], pz[:])
                nc.sync.dma_start(zt[:, c0:c0 + csz], zo[:])

            # interleave: expert 0, shared half, expert 1, shared half —
            # shared-expert matmuls fill the PE while expert-1 weights stream.
            shared_chunks = _token_chunks(B)
            nsh = len(shared_chunks)
            expert_block(0)
            for (c0, csz) in shared_chunks[:nsh // 2]:
                shared_group(c0, csz)
            for e in range(1, EPC):
                expert_block(e)
            for (c0, csz) in shared_chunks[nsh // 2:]:
                shared_group(c0, csz)

    if legalize:
        _legalize_waits(nc)
    return nc


_NC_CACHE = {}


def _pack_kblocks(mat):
    """[Ktot, F] -> [128, (Ktot/128)*F] with col block k = mat[128k:128(k+1), :]."""
    ktot, f = mat.shape
    assert ktot % 128 == 0
    return np.ascontiguousarray(
        mat.reshape(ktot // 128, 128, f).transpose(1, 0, 2).reshape(128, -1))


def prepare(x, task_id, gate_w, w1, b1, w2, b2, w3, b3,
            sw1, sb1, sw2, sb2, sw3, sb3, ow, ob):
    """Host-side routing + packing.  Returns everything needed to launch the
    device program and combine its partial outputs."""
    x = np.asarray(x, np.float32)
    f32 = lambda a: np.asarray(a, np.float32)
    gate_w, w1, b1, w2, b2, w3, b3 = map(f32, (gate_w, w1, b1, w2, b2, w3, b3))
    sw1, sb1, sw2, sb2, sw3, sb3, ow, ob = map(f32, (sw1, sb1, sw2, sb2, sw3, sb3, ow, ob))

    # ---- host gate: softmax + top-2 (the routing decision) ----
    logits = x @ gate_w.T
    logits -= logits.max(axis=1, keepdims=True)
    ex = np.exp(logits)
    scores = ex / ex.sum(axis=1, keepdims=True)            # [B, E] fp32
    order = np.argsort(-scores, axis=1, kind="stable")[:, :TOPK]   # [B, 2]

    tok_lists = []
    for e in range(E):
        sel = np.nonzero((order == e).any(axis=1))[0]
        tok_lists.append(sel)
    max_cnt = max(len(t) for t in tok_lists)
    cap = max(128, -(-max_cnt // 16) * 16)

    if cap not in _NC_CACHE:
        _NC_CACHE[cap] = _build_nc(cap)
    nc = _NC_CACHE[cap]

    # ---- pack per-core inputs (device datapath dtype) ----
    xt_p = _pack_kblocks(x.T.copy()).astype(NPDT)          # [128, KW*B]
    in_maps = []
    for c in range(NCORES):
        exps = [c * EPC + j for j in range(EPC)]
        xg_blocks, w1_bl, w3_bl, w2_bl = [], [], [], []
        bias_cols = []
        for e in exps:
            toks = tok_lists[e]
            xge = np.zeros((W, cap), np.float32)
            xge[:, :len(toks)] = x[toks].T
            xg_blocks.append(_pack_kblocks(xge).astype(NPDT))
            w1_bl.append(_pack_kblocks(w1[e].T.copy()).astype(NPDT))
            w3_bl.append(_pack_kblocks(w3[e].T.copy()).astype(NPDT))
            w2_bl.append(_pack_kblocks(w2[e].T @ ow.T).astype(NPDT))
        for e in exps:
            bias_cols.append(b1[e].reshape(MI, 128).T)     # [128, MI]
            bias_cols.append(b3[e].reshape(MI, 128).T)
        s = slice(c * SHS, (c + 1) * SHS)
        bias_cols.append(sb1[s].reshape(MS, 128).T)
        bias_cols.append(sb3[s].reshape(MS, 128).T)
        in_maps.append({
            "xt": xt_p,
            "xg": np.concatenate(xg_blocks, axis=1),
            "w1t": np.concatenate(w1_bl, axis=1),
            "w3t": np.concatenate(w3_bl, axis=1),
            "w2ot": np.concatenate(w2_bl, axis=1),
            "sw1t": _pack_kblocks(sw1[s].T.copy()).astype(NPDT),
            "sw3t": _pack_kblocks(sw3[s].T.copy()).astype(NPDT),
            "sw2ot": _pack_kblocks(sw2[:, s].T @ ow.T).astype(NPDT),
            "bias": np.ascontiguousarray(np.concatenate(bias_cols, axis=1)),
        })

    # dense combine weights [B, E] (zero except the top-2 experts per token)
    combine_w = np.zeros((B, E), np.float32)
    rows = np.arange(B)
    combine_w[rows[:, None], order] = np.take_along_axis(scores, order, axis=1)
    # analytic bias terms: sum_e combine[:,e] * (b2[e] @ ow.T)  +  sb2 @ ow.T + ob
    base = combine_w @ (b2 @ ow.T) + sb2 @ ow.T + ob

    return dict(nc=nc, cap=cap, in_maps=in_maps, tok_lists=tok_lists,
                combine_w=combine_w, base=base)


def combine(p, results):
    """Combine per-core device partials into the full [B, OUT] output."""
    cap, tok_lists, combine_w = p["cap"], p["tok_lists"], p["combine_w"]
    out = p["base"].astype(np.float32).copy()
    for c in range(NCORES):
        r = results[c]
        out += r["zt"].T
        for j in range(EPC):
            e = c * EPC + j
            toks = tok_lists[e]
            yre = r["yr"][:, j * cap: j * cap + len(toks)]  # [OUT, cnt]
            out[toks] += combine_w[toks, e][:, None] * yre.T
    return out


def kernel(x, task_id, gate_w, w1, b1, w2, b2, w3, b3,
           sw1, sb1, sw2, sb2, sw3, sb3, ow, ob):
    global LAST_RESULTS
    p = prepare(x, task_id, gate_w, w1, b1, w2, b2, w3, b3,
                sw1, sb1, sw2, sb2, sw3, sb3, ow, ob)
    res = run_bass_kernel_spmd(
        p["nc"], p["in_maps"], core_ids=list(range(NCORES)),
        trace=TRACE, **TRACE_KW)
    LAST_RESULTS = res
    return combine(p, res.results)
